# revision 12
# baseline (speedup 1.0000x reference)
"""Trainium2 Bass kernel for multi-head causal self-attention.

Problem: nn_MultiHeadSelfAttention (B=2, T=2048, D=1024, H=16, hd=64), fp32.

Sharding (8 NeuronCores, single NEFF, SPMD with per-core input slices):
  core c -> batch b = c // 4, heads h0 = (c % 4) * 4 .. h0+4  (4 heads/core).
  Each core computes QKV projection for its heads, causal flash-style
  attention (block-skipped upper triangle, no max-subtraction -- scores are
  O(+-10) so exp is safe in fp32), and a partial output projection over its
  head slice. Host sums the 4 partials per batch and adds b_proj.

All matmuls run in float32r (TF32-like, ~1.5e-4 rel err, full-rate at
free-dim >= 256). Everything on-chip between the input load and the partial
projection output.
"""

import os

import numpy as np

import concourse.bacc as bacc
import concourse.mybir as mybir
import concourse.tile as tile
from concourse import bass_utils
from concourse.bass_interp import get_hw_module
from concourse.masks import make_identity, make_upper_triangular

# Problem constants (hardcoded per contract).
D = 1024
H = 16
HD = 64
B = 2
T = 2048
NCORES = 8
NH = 4          # heads per core
QC = 512        # query-chunk width
NQC = T // QC   # 4
KT = 128        # key-tile height
NKT = T // KT   # 16
NEG_SCALE = 1.0 / np.sqrt(HD)

F32 = mybir.dt.float32
F32R = mybir.dt.float32r


def _build():
    phases = int(os.environ.get("KERNEL_PHASES", "3"))
    vbias_mm = os.environ.get("KERNEL_VBIAS_MM", "1") == "1"
    nc = bacc.Bacc("TRN2", target_bir_lowering=False, debug=False, num_devices=NCORES)

    x_d = nc.dram_tensor("x", [T, D], F32, kind="ExternalInput").ap()
    wqkv_d = nc.dram_tensor("wqkv", [3 * NH * HD, D], F32, kind="ExternalInput").ap()
    bqkv_d = nc.dram_tensor("bqkv", [3 * NH * HD], F32, kind="ExternalInput").ap()
    wproj_d = nc.dram_tensor("wproj", [D, NH * HD], F32, kind="ExternalInput").ap()
    z_d = nc.dram_tensor("zpart", [T, D], F32, kind="ExternalOutput").ap()

    with tile.TileContext(nc) as tc:
        with (
            tc.tile_pool(name="persist", bufs=1) as pp,
            tc.tile_pool(name="xt", bufs=12) as xtp,
            tc.tile_pool(name="pt", bufs=4) as ptp,
            tc.tile_pool(name="sbtmp", bufs=3) as sbtmp,
            tc.tile_pool(name="zout", bufs=2) as zoutp,
            tc.tile_pool(name="st", bufs=2, space="PSUM") as stp,
            tc.tile_pool(name="yaug", bufs=2, space="PSUM") as yaugp,
            tc.tile_pool(name="gen", bufs=2, space="PSUM") as genp,
        ):
            # ---------------- Phase 0: weights / constants ----------------
            wq_t = pp.tile([128, 8 * 2 * 128], F32R, tag="wq_t")
            wk_t = pp.tile([128, 8 * 2 * 128], F32R, tag="wk_t")
            wv_t = pp.tile([128, 8 * 256], F32R, tag="wv_t")
            wp_t = pp.tile([128, 2 * 1024], F32R, tag="wp_t")
            for kc in range(8):
                ds = slice(kc * 128, (kc + 1) * 128)
                nc.sync.dma_start(
                    wq_t[:, kc * 256:(kc + 1) * 256],
                    wqkv_d[0:256, ds].rearrange("e d -> d e").bitcast(F32R),
                )
                nc.sync.dma_start(
                    wk_t[:, kc * 256:(kc + 1) * 256],
                    wqkv_d[256:512, ds].rearrange("e d -> d e").bitcast(F32R),
                )
                nc.sync.dma_start(
                    wv_t[:, kc * 256:(kc + 1) * 256],
                    wqkv_d[512:768, ds].rearrange("e d -> d e").bitcast(F32R),
                )
            for ci in range(2):
                nc.sync.dma_start(
                    wp_t[:, ci * 1024:(ci + 1) * 1024],
                    wproj_d[:, ci * 128:(ci + 1) * 128].rearrange("o i -> i o").bitcast(F32R),
                )

            bias_sb = pp.tile([128, 4], F32, tag="bias")  # q e-tiles 0,1; k e-tiles 2,3
            for i in range(4):
                nc.sync.dma_start(
                    bias_sb[:, i:i + 1],
                    bqkv_d[i * 128:(i + 1) * 128].rearrange("(e o) -> e o", o=1),
                )
            bv_sb = pp.tile([1, 256], F32R, tag="bv")
            nc.sync.dma_start(
                bv_sb[:], bqkv_d[512:768].rearrange("(o e) -> o e", o=1).bitcast(F32R)
            )
            ones_f32 = pp.tile([128, 128], F32, tag="ones_f32")
            nc.vector.memset(ones_f32[:], 1.0)
            zeros_f32 = pp.tile([128, 128], F32, tag="zeros_f32")
            nc.vector.memset(zeros_f32[:], 0.0)
            ones1 = pp.tile([1, 128], F32R, tag="ones1")
            nc.vector.tensor_copy(ones1[:], ones_f32[0:1, :])

            ident = pp.tile([128, 128], F32, tag="ident")
            make_identity(nc, ident[:])
            mask01 = pp.tile([128, 128], F32, tag="mask01")
            make_upper_triangular(nc, mask01[:], val=1.0, diag=True)

            qt_sb = [pp.tile([128, T], F32R, tag=f"qt{i}", name=f"qt{i}") for i in range(2)]
            kt_sb = [pp.tile([128, T], F32R, tag=f"kt{i}", name=f"kt{i}") for i in range(2)]
            vaug = pp.tile([128, NKT * 260], F32R, tag="vaug")
            y_all = ysb = None
            if phases >= 2:
                y_all = pp.tile([128, NKT * 256], F32, tag="y_all")
            if phases >= 3:
                ysb = [pp.tile([128, T], F32R, tag=f"ysb{i}", name=f"ysb{i}") for i in range(2)]

            # ---------------- Phase 1: QKV projection ----------------
            for tcn in range(4):
                xts = []
                for kc in range(8):
                    xt = xtp.tile([128, QC], F32R, tag="xt")
                    nc.sync.dma_start(
                        xt[:],
                        x_d[tcn * QC:(tcn + 1) * QC, kc * 128:(kc + 1) * 128]
                        .rearrange("t d -> d t")
                        .bitcast(F32R),
                    )
                    xts.append(xt)
                # Q^T, K^T: [e_tile 128, t 512] accumulating over 8 d-chunks
                for w_t, dst, bcol in ((wq_t, qt_sb, 0), (wk_t, kt_sb, 2)):
                    for e in range(2):
                        ps = genp.tile([128, QC], F32, tag="gen")
                        for kc in range(8):
                            nc.tensor.matmul(
                                ps[:],
                                w_t[:, kc * 256 + e * 128:kc * 256 + (e + 1) * 128],
                                xts[kc][:],
                                start=(kc == 0),
                                stop=(kc == 7),
                            )
                        nc.vector.tensor_scalar_add(
                            dst[e][:, tcn * QC:(tcn + 1) * QC],
                            ps[:],
                            bias_sb[:, bcol + e:bcol + e + 1],
                        )
                # V: [t_tile 128, e 256] + bias via ones-matmul
                for tti in range(4):
                    tt = tcn * 4 + tti
                    ps = genp.tile([128, 256], F32, tag="gen")
                    for kc in range(8):
                        nc.tensor.matmul(
                            ps[:],
                            xts[kc][:, tti * 128:(tti + 1) * 128],
                            wv_t[:, kc * 256:(kc + 1) * 256],
                            start=(kc == 0),
                            stop=(kc == 7 and not vbias_mm),
                        )
                    if vbias_mm:
                        nc.tensor.matmul(
                            ps[:], ones1[:, 0:128], bv_sb[:], start=False, stop=True
                        )
                    seg = vaug[:, tt * 260:(tt + 1) * 260].rearrange(
                        "p (h c) -> p h c", c=65
                    )
                    nc.vector.tensor_copy(
                        seg[:, :, 0:64],
                        ps[:].rearrange("p (h c) -> p h c", c=64),
                    )
                    nc.vector.tensor_copy(
                        seg[:, :, 64:65],
                        ones_f32[:, 0:4].rearrange("p (h c) -> p h c", c=1),
                    )

            # ---------------- Phase 2: attention ----------------
            for h in range(NH if phases >= 2 else 0):
                qth = qt_sb[h // 2]
                kth = kt_sb[h // 2]
                po = (h % 2) * 64
                for qc in range(NQC):
                    nkt = 4 * qc + 4
                    ya = yaugp.tile([65, QC], F32, tag="yaug")
                    for kp in range(nkt // 2):
                        st = stp.tile([128, 2 * QC], F32, tag="st")
                        pt = ptp.tile([128, 2 * QC], F32R, tag="pt")
                        f0s = []
                        for j in (0, 1):
                            kti = kp * 2 + j
                            d0 = kti * 128 - qc * QC  # k0 - q0
                            f0 = 256 if d0 >= 256 else 0
                            f0s.append(f0)
                            nc.tensor.matmul(
                                st[:, j * QC + f0:(j + 1) * QC],
                                kth[po:po + 64, kti * 128:(kti + 1) * 128],
                                qth[po:po + 64, qc * QC + f0:(qc + 1) * QC],
                                start=True,
                                stop=True,
                            )
                        # exp (ACT), fused softmax scale
                        if f0s == [0, 0]:
                            nc.scalar.activation(
                                pt[:], st[:],
                                mybir.ActivationFunctionType.Exp,
                                scale=float(NEG_SCALE),
                            )
                        else:
                            assert f0s == [256, 256]
                            stv = st[:].rearrange("p (j c) -> p j c", c=QC)
                            ptv = pt[:].rearrange("p (j c) -> p j c", c=QC)
                            nc.scalar.activation(
                                ptv[:, :, 256:QC], stv[:, :, 256:QC],
                                mybir.ActivationFunctionType.Exp,
                                scale=float(NEG_SCALE),
                            )
                        # causal fixups on diagonal blocks + AV
                        for j in (0, 1):
                            kti = kp * 2 + j
                            d0 = kti * 128 - qc * QC
                            f0 = f0s[j]
                            if d0 >= 0:
                                if d0 > f0:
                                    nc.vector.tensor_copy(
                                        pt[:, j * QC + f0:j * QC + d0],
                                        zeros_f32[:, 0:d0 - f0],
                                    )
                                nc.vector.tensor_mul(
                                    pt[:, j * QC + d0:j * QC + d0 + 128],
                                    pt[:, j * QC + d0:j * QC + d0 + 128],
                                    mask01[:],
                                )
                            nc.tensor.matmul(
                                ya[0:65, f0:QC],
                                vaug[:, kti * 260 + h * 65:kti * 260 + (h + 1) * 65],
                                pt[:, j * QC + f0:(j + 1) * QC],
                                start=(kti == 0),
                                stop=(kti == nkt - 1),
                            )
                    # transpose y_aug^T back to [q, 65], normalize by row-sums
                    ya_sb = sbtmp.tile([65, QC], F32, tag="ya_sb")
                    nc.vector.tensor_copy(ya_sb[:], ya[:])
                    for sub in range(4):
                        t1 = genp.tile([128, 65], F32, tag="gen")
                        nc.tensor.matmul(
                            t1[:],
                            ya_sb[0:65, sub * 128:(sub + 1) * 128],
                            ident[0:65, 0:65],
                            is_transpose=True,
                        )
                        rec = sbtmp.tile([128, 1], F32, tag="rec")
                        nc.vector.reciprocal(rec[:], t1[:, 64:65])
                        tt = qc * 4 + sub
                        nc.vector.tensor_scalar_mul(
                            y_all[:, tt * 256 + h * 64:tt * 256 + (h + 1) * 64],
                            t1[:, 0:64],
                            rec[:],
                        )

            # ---------------- Phase 3: output projection ----------------
            if phases < 3:
                zdbg = zoutp.tile([128, QC], F32, tag="zs", name="zdbg")
                nc.vector.tensor_copy(zdbg[:, 0:128], ones_f32[:])
                nc.sync.dma_start(z_d[0:128, 0:QC], zdbg[:])
            for tt in range(NKT if phases >= 3 else 0):
                for ci in range(2):
                    t2 = genp.tile([128, 128], F32, tag="gen")
                    nc.tensor.matmul(
                        t2[:],
                        y_all[:, tt * 256 + ci * 128:tt * 256 + (ci + 1) * 128],
                        ident[:],
                        is_transpose=True,
                    )
                    nc.vector.tensor_copy(ysb[ci][:, tt * 128:(tt + 1) * 128], t2[:])
            for tt in range(NKT if phases >= 3 else 0):
                for dc in range(2):
                    zp = genp.tile([128, QC], F32, tag="gen")
                    for ci in range(2):
                        nc.tensor.matmul(
                            zp[:],
                            ysb[ci][:, tt * 128:(tt + 1) * 128],
                            wp_t[:, ci * 1024 + dc * QC:ci * 1024 + (dc + 1) * QC],
                            start=(ci == 0),
                            stop=(ci == 1),
                        )
                    zs = zoutp.tile([128, QC], F32, tag="zs")
                    nc.scalar.copy(zs[:], zp[:])
                    nc.sync.dma_start(
                        z_d[tt * 128:(tt + 1) * 128, dc * QC:(dc + 1) * QC], zs[:]
                    )

    nc.compile()
    nc.m = get_hw_module(nc.m)
    return nc


_NC_CACHE = None


def _get_nc():
    global _NC_CACHE
    if _NC_CACHE is None:
        _NC_CACHE = _build()
    return _NC_CACHE


def _in_maps(x, w_qkv, b_qkv, w_proj):
    x = np.ascontiguousarray(x, dtype=np.float32)
    w_qkv = np.ascontiguousarray(w_qkv, dtype=np.float32)
    b_qkv = np.ascontiguousarray(b_qkv, dtype=np.float32)
    w_proj = np.ascontiguousarray(w_proj, dtype=np.float32)
    maps = []
    for c in range(NCORES):
        b = c // 4
        h0 = (c % 4) * NH
        r0 = h0 * HD
        rows = np.r_[r0:r0 + 256, D + r0:D + r0 + 256, 2 * D + r0:2 * D + r0 + 256]
        maps.append(
            {
                "x": np.ascontiguousarray(x[b]),
                "wqkv": np.ascontiguousarray(w_qkv[rows]),
                "bqkv": np.ascontiguousarray(b_qkv[rows]),
                "wproj": np.ascontiguousarray(w_proj[:, r0:r0 + 256]),
            }
        )
    return maps


def _run(inputs, trace=False, **kw):
    nc = _get_nc()
    maps = _in_maps(
        inputs["x"], inputs["w_qkv"], inputs["b_qkv"], inputs["w_proj"]
    )
    return bass_utils.run_bass_kernel_spmd(
        nc, maps, core_ids=list(range(NCORES)), trace=trace, **kw
    )


def kernel(x, attn_mask, w_qkv, b_qkv, w_proj, b_proj):
    # attn_mask is the fixed causal (lower-triangular) mask; causality is
    # implemented structurally in the kernel.
    res = _run(
        {"x": x, "w_qkv": w_qkv, "b_qkv": b_qkv, "w_proj": w_proj}
    )
    out = np.zeros((B, T, D), dtype=np.float32)
    for c in range(NCORES):
        out[c // 4] += res.results[c]["zpart"]
    out += np.asarray(b_proj, dtype=np.float32)
    return out


# revision 13
# speedup vs baseline: 6.2231x; 6.2231x over previous
"""Trainium2 Bass kernel for multi-head causal self-attention.

Problem: nn_MultiHeadSelfAttention (B=2, T=2048, D=1024, H=16, hd=64), fp32.

Sharding (8 NeuronCores, single NEFF, SPMD with per-core input slices):
  core c -> batch b = c // 4, heads h0 = (c % 4) * 4 .. h0+4  (4 heads/core).
  Each core computes QKV projection for its heads, causal flash-style
  attention (block-skipped upper triangle, no max-subtraction -- scores are
  O(+-10) so exp is safe in fp32), and a partial output projection over its
  head slice. Host sums the 4 partials per batch and adds b_proj.

All matmuls run in float32r (TF32-like, ~1.5e-4 rel err, full-rate at
free-dim >= 256). Everything on-chip between the input load and the partial
projection output.
"""

import os

import numpy as np

import concourse.bacc as bacc
import concourse.mybir as mybir
import concourse.tile as tile
from concourse import bass_utils
from concourse.bass_interp import get_hw_module
from concourse.masks import make_identity, make_upper_triangular

# Problem constants (hardcoded per contract).
D = 1024
H = 16
HD = 64
B = 2
T = 2048
NCORES = 8
NH = 4          # heads per core
QC = 512        # query-chunk width
NQC = T // QC   # 4
KT = 128        # key-tile height
NKT = T // KT   # 16
NEG_SCALE = 1.0 / np.sqrt(HD)

F32 = mybir.dt.float32
F32R = mybir.dt.float32r


def _build():
    phases = int(os.environ.get("KERNEL_PHASES", "3"))
    vbias_mm = os.environ.get("KERNEL_VBIAS_MM", "1") == "1"
    nc = bacc.Bacc("TRN2", target_bir_lowering=False, debug=False, num_devices=NCORES)

    x_d = nc.dram_tensor("x", [T, D], F32, kind="ExternalInput").ap()
    wqkv_d = nc.dram_tensor("wqkv", [3 * NH * HD, D], F32, kind="ExternalInput").ap()
    bqkv_d = nc.dram_tensor("bqkv", [3 * NH * HD], F32, kind="ExternalInput").ap()
    wproj_d = nc.dram_tensor("wproj", [D, NH * HD], F32, kind="ExternalInput").ap()
    z_d = nc.dram_tensor("zpart", [T, D], F32, kind="ExternalOutput").ap()

    with tile.TileContext(nc) as tc:
        with (
            tc.tile_pool(name="persist", bufs=1) as pp,
            tc.tile_pool(name="xt", bufs=12) as xtp,
            tc.tile_pool(name="stg", bufs=4) as stgp,
            tc.tile_pool(name="pt", bufs=4) as ptp,
            tc.tile_pool(name="sbtmp", bufs=3) as sbtmp,
            tc.tile_pool(name="zout", bufs=2) as zoutp,
            tc.tile_pool(name="st", bufs=2, space="PSUM") as stp,
            tc.tile_pool(name="yaug", bufs=2, space="PSUM") as yaugp,
            tc.tile_pool(name="tpp", bufs=2, space="PSUM") as tpp,
        ):
            # ---------------- Phase 0: weights / constants ----------------
            wq_t = pp.tile([128, 8 * 2 * 128], F32R, tag="wq_t")
            wk_t = pp.tile([128, 8 * 2 * 128], F32R, tag="wk_t")
            wv_t = pp.tile([128, 8 * 256], F32R, tag="wv_t")
            wp_t = pp.tile([128, 2 * 1024], F32R, tag="wp_t")
            ones_f32 = pp.tile([128, 128], F32, tag="ones_f32")
            nc.vector.memset(ones_f32[:], 1.0)
            zeros_f32 = pp.tile([128, 128], F32, tag="zeros_f32")
            nc.vector.memset(zeros_f32[:], 0.0)
            ones1 = pp.tile([1, 128], F32R, tag="ones1")
            nc.vector.tensor_copy(ones1[:], ones_f32[0:1, :])
            ident = pp.tile([128, 128], F32, tag="ident")
            make_identity(nc, ident[:])
            mask01 = pp.tile([128, 128], F32, tag="mask01")
            make_upper_triangular(nc, mask01[:], val=1.0, diag=True)

            # Loaded contiguous, transposed on-chip (PE), cast to f32r (DVE).
            # (A strided "transposed" DMA explodes into 4-byte descriptors.)
            wload = []
            for wt_dst, row0, et in (
                (wq_t, 0, 0), (wq_t, 0, 1),
                (wk_t, 256, 0), (wk_t, 256, 1),
                (wv_t, 512, 0), (wv_t, 512, 1),
            ):
                wload.append((wt_dst, row0, et))
            for wt_dst, row0, et in wload:
                stg = stgp.tile([128, 1024], F32, tag="stg")
                r = row0 + et * 128
                nc.sync.dma_start(stg[:], wqkv_d[r:r + 128, :])
                for kc in range(8):
                    tp = tpp.tile([128, 128], F32, tag="tpp")
                    nc.tensor.matmul(
                        tp[:], stg[:, kc * 128:(kc + 1) * 128], ident[:],
                        is_transpose=True,
                    )
                    nc.vector.tensor_copy(
                        wt_dst[:, kc * 256 + et * 128:kc * 256 + (et + 1) * 128],
                        tp[:],
                    )
            for ot in range(8):
                stg = stgp.tile([128, 1024], F32, tag="stg")
                nc.sync.dma_start(stg[:, 0:256], wproj_d[ot * 128:(ot + 1) * 128, :])
                for ci in range(2):
                    tp = tpp.tile([128, 128], F32, tag="tpp")
                    nc.tensor.matmul(
                        tp[:], stg[:, ci * 128:(ci + 1) * 128], ident[:],
                        is_transpose=True,
                    )
                    nc.vector.tensor_copy(
                        wp_t[:, ci * 1024 + ot * 128:ci * 1024 + (ot + 1) * 128],
                        tp[:],
                    )

            bias_sb = pp.tile([128, 4], F32, tag="bias")  # q e-tiles 0,1; k e-tiles 2,3
            for i in range(4):
                nc.sync.dma_start(
                    bias_sb[:, i:i + 1],
                    bqkv_d[i * 128:(i + 1) * 128].rearrange("(e o) -> e o", o=1),
                )
            bv_sb = pp.tile([1, 256], F32R, tag="bv")
            nc.sync.dma_start(
                bv_sb[:], bqkv_d[512:768].rearrange("(o e) -> o e", o=1).bitcast(F32R)
            )

            qt_sb = [pp.tile([128, T], F32R, tag=f"qt{i}", name=f"qt{i}") for i in range(2)]
            kt_sb = [pp.tile([128, T], F32R, tag=f"kt{i}", name=f"kt{i}") for i in range(2)]
            vaug = pp.tile([128, NKT * 260], F32R, tag="vaug")
            y_all = ysb = None
            if phases >= 2:
                y_all = pp.tile([128, NKT * 256], F32, tag="y_all")
            if phases >= 3:
                ysb = [pp.tile([128, T], F32R, tag=f"ysb{i}", name=f"ysb{i}") for i in range(2)]

            # ---------------- Phase 1: QKV projection ----------------
            for tcn in range(4):
                xts = []
                for kc in range(8):
                    xt = xtp.tile([128, QC], F32R, tag="xt", name=f"xt{tcn}_{kc}")
                    xts.append(xt)
                for tti in range(4):
                    stg = stgp.tile([128, 1024], F32, tag="stg")
                    t0 = (tcn * 4 + tti) * 128
                    nc.sync.dma_start(stg[:], x_d[t0:t0 + 128, :])
                    for kc in range(8):
                        tp = tpp.tile([128, 128], F32, tag="tpp")
                        nc.tensor.matmul(
                            tp[:], stg[:, kc * 128:(kc + 1) * 128], ident[:],
                            is_transpose=True,
                        )
                        nc.vector.tensor_copy(
                            xts[kc][:, tti * 128:(tti + 1) * 128], tp[:]
                        )
                # Q^T, K^T: [e_tile 128, t 512] accumulating over 8 d-chunks
                for w_t, dst, bcol in ((wq_t, qt_sb, 0), (wk_t, kt_sb, 2)):
                    for e in range(2):
                        ps = stp.tile([128, QC], F32, tag="st")
                        for kc in range(8):
                            nc.tensor.matmul(
                                ps[:],
                                w_t[:, kc * 256 + e * 128:kc * 256 + (e + 1) * 128],
                                xts[kc][:],
                                start=(kc == 0),
                                stop=(kc == 7),
                            )
                        nc.vector.tensor_scalar_add(
                            dst[e][:, tcn * QC:(tcn + 1) * QC],
                            ps[:],
                            bias_sb[:, bcol + e:bcol + e + 1],
                        )
                # V: [t_tile 128, e 256] + bias via ones-matmul
                for tti in range(4):
                    tt = tcn * 4 + tti
                    ps = yaugp.tile([128, 256], F32, tag="yaug")
                    for kc in range(8):
                        nc.tensor.matmul(
                            ps[:],
                            xts[kc][:, tti * 128:(tti + 1) * 128],
                            wv_t[:, kc * 256:(kc + 1) * 256],
                            start=(kc == 0),
                            stop=(kc == 7 and not vbias_mm),
                        )
                    if vbias_mm:
                        nc.tensor.matmul(
                            ps[:], ones1[:, 0:128], bv_sb[:], start=False, stop=True
                        )
                    seg = vaug[:, tt * 260:(tt + 1) * 260].rearrange(
                        "p (h c) -> p h c", c=65
                    )
                    nc.vector.tensor_copy(
                        seg[:, :, 0:64],
                        ps[:].rearrange("p (h c) -> p h c", c=64),
                    )
                    nc.vector.tensor_copy(
                        seg[:, :, 64:65],
                        ones_f32[:, 0:4].rearrange("p (h c) -> p h c", c=1),
                    )

            # ---------------- Phase 2: attention ----------------
            for h in range(NH if phases >= 2 else 0):
                qth = qt_sb[h // 2]
                kth = kt_sb[h // 2]
                po = (h % 2) * 64
                for qc in range(NQC):
                    nkt = 4 * qc + 4
                    ya = yaugp.tile([65, QC], F32, tag="yaug")
                    for kp in range(nkt // 2):
                        st = stp.tile([128, 2 * QC], F32, tag="st")
                        pt = ptp.tile([128, 2 * QC], F32R, tag="pt")
                        f0s = []
                        for j in (0, 1):
                            kti = kp * 2 + j
                            d0 = kti * 128 - qc * QC  # k0 - q0
                            f0 = 256 if d0 >= 256 else 0
                            f0s.append(f0)
                            nc.tensor.matmul(
                                st[:, j * QC + f0:(j + 1) * QC],
                                kth[po:po + 64, kti * 128:(kti + 1) * 128],
                                qth[po:po + 64, qc * QC + f0:(qc + 1) * QC],
                                start=True,
                                stop=True,
                            )
                        # exp (ACT), fused softmax scale
                        if f0s == [0, 0]:
                            nc.scalar.activation(
                                pt[:], st[:],
                                mybir.ActivationFunctionType.Exp,
                                scale=float(NEG_SCALE),
                            )
                        else:
                            assert f0s == [256, 256]
                            stv = st[:].rearrange("p (j c) -> p j c", c=QC)
                            ptv = pt[:].rearrange("p (j c) -> p j c", c=QC)
                            nc.scalar.activation(
                                ptv[:, :, 256:QC], stv[:, :, 256:QC],
                                mybir.ActivationFunctionType.Exp,
                                scale=float(NEG_SCALE),
                            )
                        # causal fixups on diagonal blocks + AV
                        for j in (0, 1):
                            kti = kp * 2 + j
                            d0 = kti * 128 - qc * QC
                            f0 = f0s[j]
                            if d0 >= 0:
                                if d0 > f0:
                                    nc.vector.tensor_copy(
                                        pt[:, j * QC + f0:j * QC + d0],
                                        zeros_f32[:, 0:d0 - f0],
                                    )
                                nc.vector.tensor_mul(
                                    pt[:, j * QC + d0:j * QC + d0 + 128],
                                    pt[:, j * QC + d0:j * QC + d0 + 128],
                                    mask01[:],
                                )
                            nc.tensor.matmul(
                                ya[0:65, f0:QC],
                                vaug[:, kti * 260 + h * 65:kti * 260 + (h + 1) * 65],
                                pt[:, j * QC + f0:(j + 1) * QC],
                                start=(kti == 0),
                                stop=(kti == nkt - 1),
                            )
                    # transpose y_aug^T back to [q, 65], normalize by row-sums
                    ya_sb = sbtmp.tile([65, QC], F32, tag="ya_sb")
                    nc.vector.tensor_copy(ya_sb[:], ya[:])
                    for sub in range(4):
                        t1 = tpp.tile([128, 65], F32, tag="tpp")
                        nc.tensor.matmul(
                            t1[:],
                            ya_sb[0:65, sub * 128:(sub + 1) * 128],
                            ident[0:65, 0:65],
                            is_transpose=True,
                        )
                        rec = sbtmp.tile([128, 1], F32, tag="rec")
                        nc.vector.reciprocal(rec[:], t1[:, 64:65])
                        tt = qc * 4 + sub
                        nc.vector.tensor_scalar_mul(
                            y_all[:, tt * 256 + h * 64:tt * 256 + (h + 1) * 64],
                            t1[:, 0:64],
                            rec[:],
                        )

            # ---------------- Phase 3: output projection ----------------
            if phases < 3:
                zdbg = zoutp.tile([128, QC], F32, tag="zs", name="zdbg")
                nc.vector.tensor_copy(zdbg[:, 0:128], ones_f32[:])
                nc.sync.dma_start(z_d[0:128, 0:QC], zdbg[:])
            for tt in range(NKT if phases >= 3 else 0):
                for ci in range(2):
                    t2 = tpp.tile([128, 128], F32, tag="tpp")
                    nc.tensor.matmul(
                        t2[:],
                        y_all[:, tt * 256 + ci * 128:tt * 256 + (ci + 1) * 128],
                        ident[:],
                        is_transpose=True,
                    )
                    nc.vector.tensor_copy(ysb[ci][:, tt * 128:(tt + 1) * 128], t2[:])
            for tt in range(NKT if phases >= 3 else 0):
                for dc in range(2):
                    zp = stp.tile([128, QC], F32, tag="st")
                    for ci in range(2):
                        nc.tensor.matmul(
                            zp[:],
                            ysb[ci][:, tt * 128:(tt + 1) * 128],
                            wp_t[:, ci * 1024 + dc * QC:ci * 1024 + (dc + 1) * QC],
                            start=(ci == 0),
                            stop=(ci == 1),
                        )
                    zs = zoutp.tile([128, QC], F32, tag="zs")
                    nc.scalar.copy(zs[:], zp[:])
                    nc.sync.dma_start(
                        z_d[tt * 128:(tt + 1) * 128, dc * QC:(dc + 1) * QC], zs[:]
                    )

    nc.compile()
    nc.m = get_hw_module(nc.m)
    return nc


_NC_CACHE = None


def _get_nc():
    global _NC_CACHE
    if _NC_CACHE is None:
        _NC_CACHE = _build()
    return _NC_CACHE


def _in_maps(x, w_qkv, b_qkv, w_proj):
    x = np.ascontiguousarray(x, dtype=np.float32)
    w_qkv = np.ascontiguousarray(w_qkv, dtype=np.float32)
    b_qkv = np.ascontiguousarray(b_qkv, dtype=np.float32)
    w_proj = np.ascontiguousarray(w_proj, dtype=np.float32)
    maps = []
    for c in range(NCORES):
        b = c // 4
        h0 = (c % 4) * NH
        r0 = h0 * HD
        rows = np.r_[r0:r0 + 256, D + r0:D + r0 + 256, 2 * D + r0:2 * D + r0 + 256]
        maps.append(
            {
                "x": np.ascontiguousarray(x[b]),
                "wqkv": np.ascontiguousarray(w_qkv[rows]),
                "bqkv": np.ascontiguousarray(b_qkv[rows]),
                "wproj": np.ascontiguousarray(w_proj[:, r0:r0 + 256]),
            }
        )
    return maps


def _run(inputs, trace=False, **kw):
    nc = _get_nc()
    maps = _in_maps(
        inputs["x"], inputs["w_qkv"], inputs["b_qkv"], inputs["w_proj"]
    )
    return bass_utils.run_bass_kernel_spmd(
        nc, maps, core_ids=list(range(NCORES)), trace=trace, **kw
    )


def kernel(x, attn_mask, w_qkv, b_qkv, w_proj, b_proj):
    # attn_mask is the fixed causal (lower-triangular) mask; causality is
    # implemented structurally in the kernel.
    res = _run(
        {"x": x, "w_qkv": w_qkv, "b_qkv": b_qkv, "w_proj": w_proj}
    )
    out = np.zeros((B, T, D), dtype=np.float32)
    for c in range(NCORES):
        out[c // 4] += res.results[c]["zpart"]
    out += np.asarray(b_proj, dtype=np.float32)
    return out


# revision 14
# speedup vs baseline: 6.7141x; 1.0789x over previous
"""Trainium2 Bass kernel for multi-head causal self-attention.

Problem: nn_MultiHeadSelfAttention (B=2, T=2048, D=1024, H=16, hd=64), fp32.

Sharding (8 NeuronCores, single NEFF, SPMD with per-core input slices):
  core c -> batch b = c // 4, heads h0 = (c % 4) * 4 .. h0+4  (4 heads/core).
  Each core computes the QKV projection for its heads, causal flash-style
  attention (upper-triangle blocks skipped; no max-subtraction -- scores are
  O(+-10) so exp is safe in fp32), and a partial output projection over its
  head slice. The host sums the 4 partials per batch and adds b_proj.

All matmuls run in float32r (~1.5e-4 rel err, full rate at free-dim >= 256).
Inputs are DMA'd contiguously and transposed on-chip with PE-transpose (a
strided "transposed" DMA degenerates to 4-byte descriptors). The t-chunk
loop interleaves QKV projection, attention, and the output projection so the
PE stays dense and HAM-warm.
"""

import os

import numpy as np

import concourse.bacc as bacc
import concourse.mybir as mybir
import concourse.tile as tile
from concourse import bass_utils
from concourse.bass_interp import get_hw_module
from concourse.masks import make_identity, make_upper_triangular

# Problem constants (hardcoded per contract).
D = 1024
H = 16
HD = 64
B = 2
T = 2048
NCORES = 8
NH = 4          # heads per core
QC = 512        # query-chunk width
NQC = T // QC   # 4
NKT = T // 128  # 16
SM_SCALE = 1.0 / np.sqrt(HD)

F32 = mybir.dt.float32
F32R = mybir.dt.float32r


def _build():
    nc = bacc.Bacc("TRN2", target_bir_lowering=False, debug=False, num_devices=NCORES)

    x_d = nc.dram_tensor("x", [T, D], F32, kind="ExternalInput").ap()
    wqkv_d = nc.dram_tensor("wqkv", [3 * NH * HD, D], F32, kind="ExternalInput").ap()
    bqkv_d = nc.dram_tensor("bqkv", [3 * NH * HD], F32, kind="ExternalInput").ap()
    wproj_d = nc.dram_tensor("wproj", [D, NH * HD], F32, kind="ExternalInput").ap()
    z_d = nc.dram_tensor("zpart", [T, D], F32, kind="ExternalOutput").ap()

    with tile.TileContext(nc) as tc:
        with (
            tc.tile_pool(name="persist", bufs=1) as pp,
            tc.tile_pool(name="xt", bufs=12) as xtp,
            tc.tile_pool(name="stg", bufs=4) as stgp,
            tc.tile_pool(name="pt", bufs=4) as ptp,
            tc.tile_pool(name="sbtmp", bufs=3) as sbtmp,
            tc.tile_pool(name="zout", bufs=2) as zoutp,
            tc.tile_pool(name="st", bufs=2, space="PSUM") as stp,
            tc.tile_pool(name="yaug", bufs=2, space="PSUM") as yaugp,
            tc.tile_pool(name="tpp", bufs=2, space="PSUM") as tpp,
        ):
            # ---- constants ----
            ones_f32 = pp.tile([128, 128], F32, tag="ones_f32")
            nc.vector.memset(ones_f32[:], 1.0)
            zeros_f32 = pp.tile([128, 128], F32, tag="zeros_f32")
            nc.vector.memset(zeros_f32[:], 0.0)
            ones1 = pp.tile([1, 128], F32R, tag="ones1")
            nc.vector.tensor_copy(ones1[:], ones_f32[0:1, :])
            ident = pp.tile([128, 128], F32, tag="ident")
            make_identity(nc, ident[:])
            mask01 = pp.tile([128, 128], F32, tag="mask01")
            make_upper_triangular(nc, mask01[:], val=1.0, diag=True)

            # ---- PE warmup: dense dummy matmuls while initial DMAs land ----
            warm = pp.tile([128, 512], F32R, tag="warm")
            for i in range(4):
                nc.vector.tensor_copy(
                    warm[:, i * 128:(i + 1) * 128], zeros_f32[:]
                )
            for i in range(30):
                wps = stp.tile([128, 512], F32, tag="st", name=f"warmps{i}")
                nc.tensor.matmul(
                    wps[:], warm[:, 0:128], warm[:], start=True, stop=True
                )

            # ---- persistent tensors ----
            wq_t = pp.tile([128, 8 * 256], F32R, tag="wq_t")
            wk_t = pp.tile([128, 8 * 256], F32R, tag="wk_t")
            wv_t = pp.tile([128, 8 * 256], F32R, tag="wv_t")
            wp_t = pp.tile([128, 2 * 1024], F32R, tag="wp_t")
            qt_sb = [pp.tile([128, T], F32R, tag=f"qt{i}", name=f"qt{i}") for i in range(2)]
            kt_sb = [pp.tile([128, T], F32R, tag=f"kt{i}", name=f"kt{i}") for i in range(2)]
            vaug = pp.tile([128, NKT * 260], F32R, tag="vaug")
            y_all = pp.tile([128, NKT * 256], F32, tag="y_all")
            ysb = [pp.tile([128, T], F32R, tag=f"ysb{i}", name=f"ysb{i}") for i in range(2)]

            # ---- weights: contiguous load + PE transpose + f32r cast ----
            for wt_dst, row0, et in (
                (wq_t, 0, 0), (wq_t, 0, 1),
                (wk_t, 256, 0), (wk_t, 256, 1),
                (wv_t, 512, 0), (wv_t, 512, 1),
            ):
                stg = stgp.tile([128, 1024], F32, tag="stg")
                r = row0 + et * 128
                nc.sync.dma_start(stg[:], wqkv_d[r:r + 128, :])
                for kc in range(8):
                    tp = tpp.tile([128, 128], F32, tag="tpp")
                    nc.tensor.matmul(
                        tp[:], stg[:, kc * 128:(kc + 1) * 128], ident[:],
                        is_transpose=True,
                    )
                    nc.vector.tensor_copy(
                        wt_dst[:, kc * 256 + et * 128:kc * 256 + (et + 1) * 128],
                        tp[:],
                    )
            for ot in range(8):
                stg = stgp.tile([128, 1024], F32, tag="stg")
                nc.sync.dma_start(stg[:, 0:256], wproj_d[ot * 128:(ot + 1) * 128, :])
                for ci in range(2):
                    tp = tpp.tile([128, 128], F32, tag="tpp")
                    nc.tensor.matmul(
                        tp[:], stg[:, ci * 128:(ci + 1) * 128], ident[:],
                        is_transpose=True,
                    )
                    nc.vector.tensor_copy(
                        wp_t[:, ci * 1024 + ot * 128:ci * 1024 + (ot + 1) * 128],
                        tp[:],
                    )

            bias_sb = pp.tile([128, 4], F32, tag="bias")  # q e-tiles 0,1; k 2,3
            for i in range(4):
                nc.sync.dma_start(
                    bias_sb[:, i:i + 1],
                    bqkv_d[i * 128:(i + 1) * 128].rearrange("(e o) -> e o", o=1),
                )
            bv_sb = pp.tile([1, 256], F32R, tag="bv")
            nc.sync.dma_start(
                bv_sb[:], bqkv_d[512:768].rearrange("(o e) -> o e", o=1).bitcast(F32R)
            )

            def qkv_chunk(tcn):
                """QKV projection for t-chunk tcn: Q^T/K^T columns, V rows."""
                xts = []
                for kc in range(8):
                    xt = xtp.tile([128, QC], F32R, tag="xt", name=f"xt{tcn}_{kc}")
                    xts.append(xt)
                for tti in range(4):
                    stg = stgp.tile([128, 1024], F32, tag="stg")
                    t0 = (tcn * 4 + tti) * 128
                    nc.sync.dma_start(stg[:], x_d[t0:t0 + 128, :])
                    for kc in range(8):
                        tp = tpp.tile([128, 128], F32, tag="tpp")
                        nc.tensor.matmul(
                            tp[:], stg[:, kc * 128:(kc + 1) * 128], ident[:],
                            is_transpose=True,
                        )
                        nc.vector.tensor_copy(
                            xts[kc][:, tti * 128:(tti + 1) * 128], tp[:]
                        )
                for w_t, dst, bcol in ((wq_t, qt_sb, 0), (wk_t, kt_sb, 2)):
                    for e in range(2):
                        ps = stp.tile([128, QC], F32, tag="st")
                        for kc in range(8):
                            nc.tensor.matmul(
                                ps[:],
                                w_t[:, kc * 256 + e * 128:kc * 256 + (e + 1) * 128],
                                xts[kc][:],
                                start=(kc == 0),
                                stop=(kc == 7),
                            )
                        nc.vector.tensor_scalar_add(
                            dst[e][:, tcn * QC:(tcn + 1) * QC],
                            ps[:],
                            bias_sb[:, bcol + e:bcol + e + 1],
                        )
                for tti in range(4):
                    tt = tcn * 4 + tti
                    ps = yaugp.tile([128, 256], F32, tag="yaug")
                    for kc in range(8):
                        nc.tensor.matmul(
                            ps[:],
                            xts[kc][:, tti * 128:(tti + 1) * 128],
                            wv_t[:, kc * 256:(kc + 1) * 256],
                            start=(kc == 0),
                            stop=False,
                        )
                    nc.tensor.matmul(
                        ps[:], ones1[:, 0:128], bv_sb[:], start=False, stop=True
                    )
                    seg = vaug[:, tt * 260:(tt + 1) * 260].rearrange(
                        "p (h c) -> p h c", c=65
                    )
                    nc.vector.tensor_copy(
                        seg[:, :, 0:64],
                        ps[:].rearrange("p (h c) -> p h c", c=64),
                    )
                    nc.vector.tensor_copy(
                        seg[:, :, 64:65],
                        ones_f32[:, 0:4].rearrange("p (h c) -> p h c", c=1),
                    )

            def attention(h, qc):
                """One head x one query chunk. S^T blocks -> exp -> AV."""
                qth = qt_sb[h // 2]
                kth = kt_sb[h // 2]
                po = (h % 2) * 64
                nkt = 4 * qc + 4
                ya = yaugp.tile([65, QC], F32, tag="yaug")
                for kp in range(nkt // 2):
                    st = stp.tile([128, 2 * QC], F32, tag="st")
                    pt = ptp.tile([128, 2 * QC], F32R, tag="pt")
                    f0s = []
                    for j in (0, 1):
                        kti = kp * 2 + j
                        d0 = kti * 128 - qc * QC  # k0 - q0
                        f0 = 256 if d0 >= 256 else 0
                        f0s.append(f0)
                        nc.tensor.matmul(
                            st[:, j * QC + f0:(j + 1) * QC],
                            kth[po:po + 64, kti * 128:(kti + 1) * 128],
                            qth[po:po + 64, qc * QC + f0:(qc + 1) * QC],
                            start=True,
                            stop=True,
                        )
                    if f0s == [0, 0]:
                        nc.scalar.activation(
                            pt[:], st[:],
                            mybir.ActivationFunctionType.Exp,
                            scale=float(SM_SCALE),
                        )
                    else:
                        stv = st[:].rearrange("p (j c) -> p j c", c=QC)
                        ptv = pt[:].rearrange("p (j c) -> p j c", c=QC)
                        nc.scalar.activation(
                            ptv[:, :, 256:QC], stv[:, :, 256:QC],
                            mybir.ActivationFunctionType.Exp,
                            scale=float(SM_SCALE),
                        )
                    for j in (0, 1):
                        kti = kp * 2 + j
                        d0 = kti * 128 - qc * QC
                        f0 = f0s[j]
                        if d0 >= 0:
                            if d0 > f0:
                                nc.vector.tensor_copy(
                                    pt[:, j * QC + f0:j * QC + d0],
                                    zeros_f32[:, 0:d0 - f0],
                                )
                            nc.vector.tensor_mul(
                                pt[:, j * QC + d0:j * QC + d0 + 128],
                                pt[:, j * QC + d0:j * QC + d0 + 128],
                                mask01[:],
                            )
                        nc.tensor.matmul(
                            ya[0:65, f0:QC],
                            vaug[:, kti * 260 + h * 65:kti * 260 + (h + 1) * 65],
                            pt[:, j * QC + f0:(j + 1) * QC],
                            start=(kti == 0),
                            stop=(kti == nkt - 1),
                        )
                # transpose y_aug^T back to [q, 65]; normalize by row-sums
                ya_sb = sbtmp.tile([65, QC], F32, tag="ya_sb")
                nc.vector.tensor_copy(ya_sb[:], ya[:])
                for sub in range(4):
                    t1 = tpp.tile([128, 65], F32, tag="tpp")
                    nc.tensor.matmul(
                        t1[:],
                        ya_sb[0:65, sub * 128:(sub + 1) * 128],
                        ident[0:65, 0:65],
                        is_transpose=True,
                    )
                    rec = sbtmp.tile([128, 1], F32, tag="rec")
                    nc.vector.reciprocal(rec[:], t1[:, 64:65])
                    tt = qc * 4 + sub
                    nc.vector.tensor_scalar_mul(
                        y_all[:, tt * 256 + h * 64:tt * 256 + (h + 1) * 64],
                        t1[:, 0:64],
                        rec[:],
                    )

            def proj(tt):
                """Output projection for one t-tile: y_all -> y^T -> z."""
                for ci in range(2):
                    t2 = tpp.tile([128, 128], F32, tag="tpp")
                    nc.tensor.matmul(
                        t2[:],
                        y_all[:, tt * 256 + ci * 128:tt * 256 + (ci + 1) * 128],
                        ident[:],
                        is_transpose=True,
                    )
                    nc.vector.tensor_copy(ysb[ci][:, tt * 128:(tt + 1) * 128], t2[:])
                for dc in range(2):
                    zp = stp.tile([128, QC], F32, tag="st")
                    for ci in range(2):
                        nc.tensor.matmul(
                            zp[:],
                            ysb[ci][:, tt * 128:(tt + 1) * 128],
                            wp_t[:, ci * 1024 + dc * QC:ci * 1024 + (dc + 1) * QC],
                            start=(ci == 0),
                            stop=(ci == 1),
                        )
                    zs = zoutp.tile([128, QC], F32, tag="zs")
                    nc.scalar.copy(zs[:], zp[:])
                    nc.sync.dma_start(
                        z_d[tt * 128:(tt + 1) * 128, dc * QC:(dc + 1) * QC], zs[:]
                    )

            # ---- interleaved chunk-major schedule ----
            for tcn in range(NQC):
                qkv_chunk(tcn)
                for h in range(NH):
                    attention(h, qc=tcn)
                for tti in range(4):
                    proj(tcn * 4 + tti)

    nc.compile()
    nc.m = get_hw_module(nc.m)
    return nc


_NC_CACHE = None


def _get_nc():
    global _NC_CACHE
    if _NC_CACHE is None:
        _NC_CACHE = _build()
    return _NC_CACHE


def _in_maps(x, w_qkv, b_qkv, w_proj):
    x = np.ascontiguousarray(x, dtype=np.float32)
    w_qkv = np.ascontiguousarray(w_qkv, dtype=np.float32)
    b_qkv = np.ascontiguousarray(b_qkv, dtype=np.float32)
    w_proj = np.ascontiguousarray(w_proj, dtype=np.float32)
    maps = []
    for c in range(NCORES):
        b = c // 4
        h0 = (c % 4) * NH
        r0 = h0 * HD
        rows = np.r_[r0:r0 + 256, D + r0:D + r0 + 256, 2 * D + r0:2 * D + r0 + 256]
        maps.append(
            {
                "x": np.ascontiguousarray(x[b]),
                "wqkv": np.ascontiguousarray(w_qkv[rows]),
                "bqkv": np.ascontiguousarray(b_qkv[rows]),
                "wproj": np.ascontiguousarray(w_proj[:, r0:r0 + 256]),
            }
        )
    return maps


def _run(inputs, trace=False, **kw):
    nc = _get_nc()
    maps = _in_maps(
        inputs["x"], inputs["w_qkv"], inputs["b_qkv"], inputs["w_proj"]
    )
    return bass_utils.run_bass_kernel_spmd(
        nc, maps, core_ids=list(range(NCORES)), trace=trace, **kw
    )


def kernel(x, attn_mask, w_qkv, b_qkv, w_proj, b_proj):
    # attn_mask is the fixed causal (lower-triangular) mask; causality is
    # implemented structurally in the kernel.
    res = _run(
        {"x": x, "w_qkv": w_qkv, "b_qkv": b_qkv, "w_proj": w_proj}
    )
    out = np.zeros((B, T, D), dtype=np.float32)
    for c in range(NCORES):
        out[c // 4] += res.results[c]["zpart"]
    out += np.asarray(b_proj, dtype=np.float32)
    return out


# revision 16
# speedup vs baseline: 7.4037x; 1.1027x over previous
"""Trainium2 Bass kernel for multi-head causal self-attention.

Problem: nn_MultiHeadSelfAttention (B=2, T=2048, D=1024, H=16, hd=64), fp32.

Sharding (8 NeuronCores, single NEFF, SPMD with per-core input slices):
  core c -> batch b = c // 4, heads h0 = (c % 4) * 4 .. h0+4  (4 heads/core).
  Each core computes the QKV projection for its heads, causal flash-style
  attention (upper-triangle blocks skipped; no max-subtraction -- scores are
  O(+-10) so exp is safe in fp32), and a partial output projection over its
  head slice. The host sums the 4 partials per batch and adds b_proj.

All matmuls run in float32r (~1.5e-4 rel err, full rate at free-dim >= 256).
Inputs are DMA'd contiguously and transposed on-chip with PE-transpose (a
strided "transposed" DMA degenerates to 4-byte descriptors). Attention
processes head PAIRS: the two heads' 64-deep QK^T contractions occupy PE row
groups (0,0)/(64,0) and execute concurrently. The t-chunk loop interleaves
QKV projection, attention, and the output projection so the PE stays dense.
"""

import os

import numpy as np

import concourse.bacc as bacc
import concourse.mybir as mybir
import concourse.tile as tile
from concourse import bass_utils
from concourse.bass_interp import get_hw_module
from concourse.masks import make_identity, make_upper_triangular

# Problem constants (hardcoded per contract).
D = 1024
H = 16
HD = 64
B = 2
T = 2048
NCORES = 8
NH = 4          # heads per core
QC = 512        # query-chunk width
NQC = T // QC   # 4
NKT = T // 128  # 16
SM_SCALE = 1.0 / np.sqrt(HD)

F32 = mybir.dt.float32
F32R = mybir.dt.float32r


def _build():
    nc = bacc.Bacc("TRN2", target_bir_lowering=False, debug=False, num_devices=NCORES)

    x_d = nc.dram_tensor("x", [T, D], F32, kind="ExternalInput").ap()
    wqkv_d = nc.dram_tensor("wqkv", [3 * NH * HD, D], F32, kind="ExternalInput").ap()
    bqkv_d = nc.dram_tensor("bqkv", [3 * NH * HD], F32, kind="ExternalInput").ap()
    wproj_d = nc.dram_tensor("wproj", [D, NH * HD], F32, kind="ExternalInput").ap()
    z_d = nc.dram_tensor("zpart", [T, D], F32, kind="ExternalOutput").ap()

    with tile.TileContext(nc) as tc:
        with (
            tc.tile_pool(name="persist", bufs=1) as pp,
            tc.tile_pool(name="xt", bufs=2) as xtp,
            tc.tile_pool(name="stg", bufs=4) as stgp,
            tc.tile_pool(name="pt", bufs=3) as ptp,
            tc.tile_pool(name="sbtmp", bufs=3) as sbtmp,
            tc.tile_pool(name="zout", bufs=2) as zoutp,
            tc.tile_pool(name="st", bufs=2, space="PSUM") as stp,
            tc.tile_pool(name="yaug", bufs=2, space="PSUM") as yaugp,
            tc.tile_pool(name="gen", bufs=2, space="PSUM") as genp,
        ):
            # ---- constants ----
            ones_f32 = pp.tile([128, 128], F32, tag="ones_f32")
            nc.vector.memset(ones_f32[:], 1.0)
            zeros_f32 = pp.tile([128, 128], F32, tag="zeros_f32")
            nc.vector.memset(zeros_f32[:], 0.0)
            ones1 = pp.tile([1, 128], F32R, tag="ones1")
            nc.vector.tensor_copy(ones1[:], ones_f32[0:1, :])
            ident = pp.tile([128, 128], F32, tag="ident")
            make_identity(nc, ident[:])
            mask01 = pp.tile([128, 128], F32, tag="mask01")
            make_upper_triangular(nc, mask01[:], val=1.0, diag=True)

            # ---- PE warmup: dense dummy matmuls while initial DMAs land ----
            warm = pp.tile([128, 512], F32R, tag="warm")
            for i in range(4):
                nc.vector.tensor_copy(
                    warm[:, i * 128:(i + 1) * 128], zeros_f32[:]
                )
            for i in range(30):
                wps = stp.tile([128, 512], F32, tag="st", name=f"warmps{i}")
                nc.tensor.matmul(
                    wps[:], warm[:, 0:128], warm[:], start=True, stop=True
                )

            # ---- persistent tensors ----
            wq_t = pp.tile([128, 8 * 256], F32R, tag="wq_t")
            wk_t = pp.tile([128, 8 * 256], F32R, tag="wk_t")
            wv_t = pp.tile([128, 8 * 256], F32R, tag="wv_t")
            wp_t = pp.tile([128, 2 * 1024], F32R, tag="wp_t")
            qt_sb = [pp.tile([128, T], F32R, tag=f"qt{i}", name=f"qt{i}") for i in range(2)]
            kt_sb = [pp.tile([128, T], F32R, tag=f"kt{i}", name=f"kt{i}") for i in range(2)]
            vaug = pp.tile([128, NKT * 260], F32R, tag="vaug")
            y_all = pp.tile([128, NKT * 256], F32, tag="y_all")
            ysb = [pp.tile([128, T], F32R, tag=f"ysb{i}", name=f"ysb{i}") for i in range(2)]

            # ---- weights: contiguous load + batched PE transpose + f32r cast ----
            for wt_dst, row0, et in (
                (wq_t, 0, 0), (wq_t, 0, 1),
                (wk_t, 256, 0), (wk_t, 256, 1),
                (wv_t, 512, 0), (wv_t, 512, 1),
            ):
                stg = stgp.tile([128, 1024], F32, tag="stg")
                r = row0 + et * 128
                nc.sync.dma_start(stg[:], wqkv_d[r:r + 128, :])
                wv = wt_dst[:].rearrange("p (kc e) -> p kc e", e=256)
                for half in range(2):
                    g = genp.tile([128, 512], F32, tag="gen")
                    for k4 in range(4):
                        kc = half * 4 + k4
                        nc.tensor.matmul(
                            g[:, k4 * 128:(k4 + 1) * 128],
                            stg[:, kc * 128:(kc + 1) * 128], ident[:],
                            is_transpose=True,
                        )
                    nc.vector.tensor_copy(
                        wv[:, half * 4:(half + 1) * 4, et * 128:(et + 1) * 128],
                        g[:].rearrange("p (kc e) -> p kc e", e=128),
                    )
            for ot in range(8):
                stg = stgp.tile([128, 1024], F32, tag="stg")
                nc.sync.dma_start(stg[:, 0:256], wproj_d[ot * 128:(ot + 1) * 128, :])
                g = genp.tile([128, 512], F32, tag="gen")
                for ci in range(2):
                    nc.tensor.matmul(
                        g[:, ci * 128:(ci + 1) * 128],
                        stg[:, ci * 128:(ci + 1) * 128], ident[:],
                        is_transpose=True,
                    )
                nc.vector.tensor_copy(
                    wp_t[:].rearrange("p (ci o) -> p ci o", o=1024)[
                        :, :, ot * 128:(ot + 1) * 128
                    ],
                    g[:, 0:256].rearrange("p (ci o) -> p ci o", o=128),
                )

            bias_sb = pp.tile([128, 4], F32, tag="bias")  # q e-tiles 0,1; k 2,3
            for i in range(4):
                nc.sync.dma_start(
                    bias_sb[:, i:i + 1],
                    bqkv_d[i * 128:(i + 1) * 128].rearrange("(e o) -> e o", o=1),
                )
            bv_sb = pp.tile([1, 256], F32R, tag="bv")
            nc.sync.dma_start(
                bv_sb[:], bqkv_d[512:768].rearrange("(o e) -> o e", o=1).bitcast(F32R)
            )

            def qkv_chunk(tcn):
                """QKV projection for t-chunk tcn: Q^T/K^T columns, V rows.

                xt_all layout: [d-partition(128) x (kc(8) * t(512))].
                """
                xt_all = xtp.tile([128, 8 * QC], F32R, tag="xt", name=f"xt{tcn}")
                xv = xt_all[:].rearrange("p (kc t) -> p kc t", t=QC)
                for tti in range(4):
                    stg = stgp.tile([128, 1024], F32, tag="stg")
                    t0 = (tcn * 4 + tti) * 128
                    nc.sync.dma_start(stg[:], x_d[t0:t0 + 128, :])
                    for half in range(2):
                        g = genp.tile([128, 512], F32, tag="gen")
                        for k4 in range(4):
                            kc = half * 4 + k4
                            nc.tensor.matmul(
                                g[:, k4 * 128:(k4 + 1) * 128],
                                stg[:, kc * 128:(kc + 1) * 128], ident[:],
                                is_transpose=True,
                            )
                        nc.vector.tensor_copy(
                            xv[:, half * 4:(half + 1) * 4,
                               tti * 128:(tti + 1) * 128],
                            g[:].rearrange("p (kc t) -> p kc t", t=128),
                        )
                for w_t, dst, bcol in ((wq_t, qt_sb, 0), (wk_t, kt_sb, 2)):
                    for e in range(2):
                        ps = genp.tile([128, QC], F32, tag="gen")
                        for kc in range(8):
                            nc.tensor.matmul(
                                ps[:],
                                w_t[:, kc * 256 + e * 128:kc * 256 + (e + 1) * 128],
                                xv[:, kc, :],
                                start=(kc == 0),
                                stop=(kc == 7),
                            )
                        nc.vector.tensor_scalar_add(
                            dst[e][:, tcn * QC:(tcn + 1) * QC],
                            ps[:],
                            bias_sb[:, bcol + e:bcol + e + 1],
                        )
                for tti in range(4):
                    tt = tcn * 4 + tti
                    ps = genp.tile([128, 256], F32, tag="gen")
                    for kc in range(8):
                        nc.tensor.matmul(
                            ps[:],
                            xv[:, kc, tti * 128:(tti + 1) * 128],
                            wv_t[:, kc * 256:(kc + 1) * 256],
                            start=(kc == 0),
                            stop=False,
                        )
                    nc.tensor.matmul(
                        ps[:], ones1[:, 0:128], bv_sb[:], start=False, stop=True
                    )
                    seg = vaug[:, tt * 260:(tt + 1) * 260].rearrange(
                        "p (h c) -> p h c", c=65
                    )
                    nc.vector.tensor_copy(
                        seg[:, :, 0:64],
                        ps[:].rearrange("p (h c) -> p h c", c=64),
                    )
                    nc.vector.tensor_copy(
                        seg[:, :, 64:65],
                        ones_f32[:, 0:4].rearrange("p (h c) -> p h c", c=1),
                    )

            def attention2(hp, qc):
                """Head pair (2*hp, 2*hp+1) x one query chunk.

                S^T for both heads lands in one [128, 1024] psum tile
                (cols 0:512 even head / 512:1024 odd head); the two QK^T
                matmuls use PE row groups (0,0) and (64,0) concurrently.
                """
                qth = qt_sb[hp]
                kth = kt_sb[hp]
                he, ho = 2 * hp, 2 * hp + 1
                nkt = 4 * qc + 4
                ya_e = yaugp.tile([65, QC], F32, tag="yaug", name=f"yae{hp}_{qc}")
                ya_o = yaugp.tile([65, QC], F32, tag="yaug", name=f"yao{hp}_{qc}")
                for kti in range(nkt):
                    d0 = kti * 128 - qc * QC  # k0 - q0
                    f0 = 256 if d0 >= 256 else 0
                    st = stp.tile([128, 2 * QC], F32, tag="st")
                    pt = ptp.tile([128, 2 * QC], F32R, tag="pt")
                    for half, po in ((0, 0), (1, 64)):
                        nc.tensor.matmul(
                            st[:, half * QC + f0:(half + 1) * QC],
                            kth[po:po + 64, kti * 128:(kti + 1) * 128],
                            qth[po:po + 64, qc * QC + f0:(qc + 1) * QC],
                            start=True,
                            stop=True,
                        )
                    if f0 == 0:
                        nc.scalar.activation(
                            pt[:], st[:],
                            mybir.ActivationFunctionType.Exp,
                            scale=float(SM_SCALE),
                        )
                    else:
                        stv = st[:].rearrange("p (j c) -> p j c", c=QC)
                        ptv = pt[:].rearrange("p (j c) -> p j c", c=QC)
                        nc.scalar.activation(
                            ptv[:, :, 256:QC], stv[:, :, 256:QC],
                            mybir.ActivationFunctionType.Exp,
                            scale=float(SM_SCALE),
                        )
                    if d0 >= 0:
                        for half in range(2):
                            if d0 > f0:
                                nc.vector.tensor_copy(
                                    pt[:, half * QC + f0:half * QC + d0],
                                    zeros_f32[:, 0:d0 - f0],
                                )
                            nc.vector.tensor_mul(
                                pt[:, half * QC + d0:half * QC + d0 + 128],
                                pt[:, half * QC + d0:half * QC + d0 + 128],
                                mask01[:],
                            )
                    for ya, h, half in ((ya_e, he, 0), (ya_o, ho, 1)):
                        nc.tensor.matmul(
                            ya[0:65, f0:QC],
                            vaug[:, kti * 260 + h * 65:kti * 260 + (h + 1) * 65],
                            pt[:, half * QC + f0:(half + 1) * QC],
                            start=(kti == 0),
                            stop=(kti == nkt - 1),
                        )
                # transpose y_aug^T back to [q, 65]; normalize by row-sums
                for ya, h in ((ya_e, he), (ya_o, ho)):
                    ya_sb = sbtmp.tile([65, QC], F32, tag="ya_sb")
                    nc.vector.tensor_copy(ya_sb[:], ya[:])
                    for sub in range(4):
                        t1 = genp.tile([128, 65], F32, tag="gen")
                        nc.tensor.matmul(
                            t1[:],
                            ya_sb[0:65, sub * 128:(sub + 1) * 128],
                            ident[0:65, 0:65],
                            is_transpose=True,
                        )
                        rec = sbtmp.tile([128, 1], F32, tag="rec")
                        nc.vector.reciprocal(rec[:], t1[:, 64:65])
                        tt = qc * 4 + sub
                        nc.vector.tensor_scalar_mul(
                            y_all[:, tt * 256 + h * 64:tt * 256 + (h + 1) * 64],
                            t1[:, 0:64],
                            rec[:],
                        )

            def proj(tt):
                """Output projection for one t-tile: y_all -> y^T -> z."""
                for ci in range(2):
                    t2 = genp.tile([128, 128], F32, tag="gen")
                    nc.tensor.matmul(
                        t2[:],
                        y_all[:, tt * 256 + ci * 128:tt * 256 + (ci + 1) * 128],
                        ident[:],
                        is_transpose=True,
                    )
                    nc.vector.tensor_copy(ysb[ci][:, tt * 128:(tt + 1) * 128], t2[:])
                for dc in range(2):
                    zp = genp.tile([128, QC], F32, tag="gen")
                    for ci in range(2):
                        nc.tensor.matmul(
                            zp[:],
                            ysb[ci][:, tt * 128:(tt + 1) * 128],
                            wp_t[:, ci * 1024 + dc * QC:ci * 1024 + (dc + 1) * QC],
                            start=(ci == 0),
                            stop=(ci == 1),
                        )
                    zs = zoutp.tile([128, QC], F32, tag="zs")
                    if dc == 0:
                        nc.scalar.copy(zs[:], zp[:])
                    else:
                        nc.vector.tensor_copy(zs[:], zp[:])
                    nc.sync.dma_start(
                        z_d[tt * 128:(tt + 1) * 128, dc * QC:(dc + 1) * QC], zs[:]
                    )

            # ---- interleaved chunk-major schedule ----
            for tcn in range(NQC):
                qkv_chunk(tcn)
                for hp in range(2):
                    attention2(hp, qc=tcn)
                for tti in range(4):
                    proj(tcn * 4 + tti)

    nc.compile()
    nc.m = get_hw_module(nc.m)
    return nc


_NC_CACHE = None


def _get_nc():
    global _NC_CACHE
    if _NC_CACHE is None:
        _NC_CACHE = _build()
    return _NC_CACHE


def _in_maps(x, w_qkv, b_qkv, w_proj):
    x = np.ascontiguousarray(x, dtype=np.float32)
    w_qkv = np.ascontiguousarray(w_qkv, dtype=np.float32)
    b_qkv = np.ascontiguousarray(b_qkv, dtype=np.float32)
    w_proj = np.ascontiguousarray(w_proj, dtype=np.float32)
    maps = []
    for c in range(NCORES):
        b = c // 4
        h0 = (c % 4) * NH
        r0 = h0 * HD
        rows = np.r_[r0:r0 + 256, D + r0:D + r0 + 256, 2 * D + r0:2 * D + r0 + 256]
        maps.append(
            {
                "x": np.ascontiguousarray(x[b]),
                "wqkv": np.ascontiguousarray(w_qkv[rows]),
                "bqkv": np.ascontiguousarray(b_qkv[rows]),
                "wproj": np.ascontiguousarray(w_proj[:, r0:r0 + 256]),
            }
        )
    return maps


def _run(inputs, trace=False, **kw):
    nc = _get_nc()
    maps = _in_maps(
        inputs["x"], inputs["w_qkv"], inputs["b_qkv"], inputs["w_proj"]
    )
    return bass_utils.run_bass_kernel_spmd(
        nc, maps, core_ids=list(range(NCORES)), trace=trace, **kw
    )


def kernel(x, attn_mask, w_qkv, b_qkv, w_proj, b_proj):
    # attn_mask is the fixed causal (lower-triangular) mask; causality is
    # implemented structurally in the kernel.
    res = _run(
        {"x": x, "w_qkv": w_qkv, "b_qkv": b_qkv, "w_proj": w_proj}
    )
    out = np.zeros((B, T, D), dtype=np.float32)
    for c in range(NCORES):
        out[c // 4] += res.results[c]["zpart"]
    out += np.asarray(b_proj, dtype=np.float32)
    return out
